# revision 1
# baseline (speedup 1.0000x reference)
"""Causal self-attention (B=2, T=4096, C=768, H=12) on 8 TRN2 NeuronCores.

Sharding: core c -> batch c//4, heads 3*(c%4) .. 3*(c%4)+2.  Each core is
fully independent (no collectives): it computes qkv for its 3 heads from
x[b], runs causal flash attention, and produces the partial output
projection outT = (Y_heads @ W_proj[rows]).T of shape [C, T].  The host
sums the 4 per-batch partials, transposes, and adds b_proj.

Per-core structure (all matmuls float32r):
  - qT/kT [64, T] per head via Wqk^T @ xT, kept twice with partition
    halves swapped so each S^T k-tile pair issues from PE row-group
    {0,1} and {2,3} concurrently.
  - v in natural [T, 64] orientation via x-tiles-as-lhsT, with a ones
    column riding along so the PV matmul emits softmax denominators.
  - exp on ACT with no max subtraction (logits are bounded), causal
    masking by multiplying the 4 diagonal k-tiles with 0/1 masks.
  - the next query-group's QKV matmuls are interleaved into the
    attention stream as PE filler to keep the PE HAM-warm.
"""

import os
import sys

import numpy as np

for _p in ("/opt/trn_rl_repo", "/root/.axon_site/_ro/trn_rl_repo"):
    if os.path.isdir(_p) and _p not in sys.path:
        sys.path.insert(0, _p)

from contextlib import ExitStack

import concourse.bacc as bacc
import concourse.bass as bass
import concourse.mybir as mybir
import concourse.tile as tile
from concourse.bass_utils import run_bass_kernel_spmd

F32 = mybir.dt.float32
F32R = mybir.dt.float32r
EXP = mybir.ActivationFunctionType.Exp
IS_GE = mybir.AluOpType.is_ge

B, T_FULL, C = 2, 4096, 768
H, DH = 12, 64
HPC = 3                      # heads per core
NCORES = 8
P = 128
QG = 512                     # query-group span (free dim of S^T tiles)
KT = 128                     # key tile (partition dim of S^T tiles)
NQK = 6 * DH                 # 384 rows of qkT (q,k for 3 heads)
NVP = 256                    # padded v width: v0|1|v1|1|v2|1|zeros
SCALE = 1.0 / np.sqrt(DH)

# qkT row layout as (block, local_head) pairs of 64 rows each
QK_ORDER = [("q", 0), ("q", 1), ("k", 0), ("k", 1), ("q", 2), ("k", 2)]
BLK = {"q": 0, "k": 1, "v": 2}


def r32(ap):
    return ap.bitcast(F32R)


def build_nc(t=T_FULL):
    ng = t // QG             # query groups
    nc = bacc.Bacc(None, target_bir_lowering=False)
    xT = nc.declare_dram_parameter("xT", [C, t], F32, isOutput=False)
    wqk = nc.declare_dram_parameter("wqk", [C, NQK], F32, isOutput=False)
    bqk = nc.declare_dram_parameter("bqk", [NQK, 1], F32, isOutput=False)
    wvp = nc.declare_dram_parameter("wvp", [C, NVP], F32, isOutput=False)
    bv = nc.declare_dram_parameter("bv", [HPC * DH, 1], F32, isOutput=False)
    wp = nc.declare_dram_parameter("wp", [HPC * DH, C], F32, isOutput=False)
    outT = nc.declare_dram_parameter("outT", [C, t], F32, isOutput=True)

    # qkT row layout: A=[q0|q1] B=[k0|k1] C=[q2|k2], plus half-swapped
    # duplicates A'=[q1|q0] B'=[k1|k0] C'=[k2|q2] so every head has its
    # (q, k) pair available at base partition 0 AND base partition 64.
    with tile.TileContext(nc) as tc, ExitStack() as ctx:
        const = ctx.enter_context(tc.tile_pool(name="const", bufs=1))
        qkp = ctx.enter_context(tc.tile_pool(name="qk", bufs=1))
        vp = ctx.enter_context(tc.tile_pool(name="vn", bufs=1))
        xpool = ctx.enter_context(tc.tile_pool(name="xin", bufs=1))
        ppool = ctx.enter_context(tc.tile_pool(name="pp", bufs=3))
        ytsb = ctx.enter_context(tc.tile_pool(name="ytsb", bufs=1))
        osb = ctx.enter_context(tc.tile_pool(name="osb", bufs=2))
        rrp = ctx.enter_context(tc.tile_pool(name="rr", bufs=1))
        spsum = ctx.enter_context(tc.tile_pool(name="sps", bufs=2, space="PSUM"))
        ytps = ctx.enter_context(tc.tile_pool(name="ytps", bufs=1, space="PSUM"))
        aux = ctx.enter_context(tc.tile_pool(name="aux", bufs=1, space="PSUM"))

        # ---- constants -------------------------------------------------
        wqk_sb, wvp_sb = [], []
        for k in range(6):
            tl = const.tile([P, NQK], F32R, tag=f"wqk{k}", name=f"wqk{k}")
            nc.sync.dma_start(tl[:], r32(wqk[k * P:(k + 1) * P, :]))
            wqk_sb.append(tl)
            tv = const.tile([P, NVP], F32R, tag=f"wvp{k}", name=f"wvp{k}")
            nc.sync.dma_start(tv[:], r32(wvp[k * P:(k + 1) * P, :]))
            wvp_sb.append(tv)
        b_sb = []
        for m in range(3):
            tl = const.tile([P, 1], F32, tag=f"bq{m}", name=f"bq{m}")
            nc.sync.dma_start(tl[:], bqk[m * P:(m + 1) * P, :])
            b_sb.append(tl)
        bv_sb = const.tile([P, 1], F32, tag="bv01", name="bv01")
        nc.sync.dma_start(bv_sb[:], bv[0:P, :])
        bv2_sb = const.tile([DH, 1], F32, tag="bv2", name="bv2")
        nc.sync.dma_start(bv2_sb[:], bv[P:P + DH, :])
        wp0 = const.tile([P, C], F32R, tag="wp0", name="wp0")
        wp1 = const.tile([DH, C], F32R, tag="wp1", name="wp1")
        nc.sync.dma_start(wp0[:], r32(wp[0:P, :]))
        nc.sync.dma_start(wp1[:], r32(wp[P:P + DH, :]))

        onesF = const.tile([P, 1], F32, tag="onesF", name="onesF")
        nc.vector.memset(onesF[:], 1.0)
        ones1 = const.tile([1, DH], F32R, tag="ones1", name="ones1")
        m2 = [const.tile([P, 2 * QG], F32R, tag=f"m2_{half}", name=f"m2_{half}")
              for half in range(2)]
        with tc.tile_pool(name="scratch", bufs=1) as scratch:
            onesRF = scratch.tile([1, DH], F32, tag="onesRF", name="onesRF")
            nc.vector.memset(onesRF[:], 1.0)
            nc.vector.tensor_copy(ones1[:], onesRF[:])
            # causal masks for the 4 diagonal k-tiles of each query group:
            # tile r (r=0..3) keeps col qi of row ki iff qi - ki - 128*r >= 0.
            m2F = scratch.tile([P, 2 * QG], F32, tag="m2F", name="m2F")
            for half in range(2):
                nc.gpsimd.memset(m2F[:], 1.0)
                for hh in range(2):
                    r = 2 * half + hh
                    nc.gpsimd.affine_select(
                        out=m2F[:, hh * QG:(hh + 1) * QG],
                        in_=m2F[:, hh * QG:(hh + 1) * QG],
                        compare_op=IS_GE,
                        fill=0.0,
                        base=-KT * r,
                        pattern=[[1, QG]],
                        channel_multiplier=-1,
                    )
                nc.vector.tensor_copy(m2[half][:], m2F[:])

        # ---- persistent qkT / v storage --------------------------------
        qkt = [qkp.tile([P, t], F32R, tag=f"qkt{i}", name=f"qkt{i}")
               for i in range(3)]          # A, B, C
        qkd = [qkp.tile([P, t], F32R, tag=f"qkd{i}", name=f"qkd{i}")
               for i in range(3)]          # A', B', C'
        A, Bt, Ct = qkt
        Ad, Bd, Cd = qkd
        qk0 = [(A, Bt), (Ad, Bd), (Ct, Cd)]      # base-0 (q_tile, k_tile)
        qk64 = [(Ad, Bd), (A, Bt), (Cd, Ct)]     # base-64 (q_tile, k_tile)
        # v in natural orientation: per 128-row t-tile, cols h*65..h*65+64
        # hold [v_h | ones]; built by matmul with the padded wvp whose
        # ones-columns have zero weights, overwritten with 1.0 afterwards.
        vnat = [vp.tile([P, NVP], F32R, tag=f"vn{j}", name=f"vn{j}")
                for j in range(t // P)]

        def v1ap(h, j):
            return vnat[j][:, 65 * h:65 * h + DH + 1]

        # ---- per-group QKV emission (used as PE filler) ----------------
        def emit_qkv_unit(g, u):
            """Emit filler unit u (0..9) of query group g's QKV work."""
            gs = slice(g * QG, (g + 1) * QG)
            if u == 0:
                xk = []
                for k in range(6):
                    tl = xpool.tile([P, QG], F32R, tag=f"x{k}", name=f"x{k}")
                    nc.sync.dma_start(tl[:], r32(xT[k * P:(k + 1) * P, gs]))
                    xk.append(tl)
                xtiles[g % 2] = xk
                return
            xk = xtiles[g % 2]
            if u <= 3:
                m = u - 1           # qkT m-tile (A, B, C)
                ps = aux.tile([P, QG], F32, tag="aux", name="qkps")
                for k in range(6):
                    nc.tensor.matmul(ps[:], wqk_sb[k][:, m * P:(m + 1) * P],
                                     xk[k][:], start=(k == 0), stop=(k == 5))
                nc.vector.tensor_scalar_add(qkt[m][:, gs], ps[:], b_sb[m][:])
            elif u <= 6:
                # half-swapped duplicate of m-tile u-4
                m = u - 4
                nc.scalar.copy(qkd[m][DH:, gs], qkt[m][:DH, gs])
                nc.scalar.copy(qkd[m][:DH, gs], qkt[m][DH:, gs])
            else:
                ti = u - 7          # v t-tile within the group (0..3)
                j = 4 * g + ti
                ps = aux.tile([P, NVP], F32, tag="aux", name="vnps")
                for k in range(6):
                    nc.tensor.matmul(ps[:], xk[k][:, ti * P:(ti + 1) * P],
                                     wvp_sb[k][:], start=(k == 0), stop=(k == 5))
                nc.vector.tensor_copy(vnat[j][:], ps[:])
                for h in range(HPC):
                    nc.vector.tensor_copy(vnat[j][:, 65 * h + DH:65 * h + DH + 1],
                                          onesF[:])

        N_UNITS = 11  # 1 dma + 3 qk + 3 dup + 4 v

        xtiles = [None, None]
        # prologue: group 0's QKV
        for u in range(N_UNITS):
            emit_qkv_unit(0, u)

        # ---- fused attention + next-group QKV filler -------------------
        for g in range(ng):
            gs = slice(g * QG, (g + 1) * QG)
            npairs = 2 * (g + 1)
            yt0 = ytsb.tile([P, QG], F32R, tag="yt0", name="yt0")
            yt1 = ytsb.tile([DH, QG], F32R, tag="yt1", name="yt1")
            yt = [ytps.tile([DH + 1, QG], F32, tag=f"yt{h}",
                            name=f"yt{h}") for h in range(HPC)]
            # spread the next group's QKV units across this group's
            # attention units
            n_att = npairs * HPC
            fill = []
            if g + 1 < ng:
                fill = [(g + 1, u) for u in range(N_UNITS)]
            nf_total = len(fill)
            next_fill = 0
            for pr in range(npairs):
                j0, j1 = 2 * pr, 2 * pr + 1
                for h in range(HPC):
                    ui = pr * HPC + h
                    q0t, k0t = qk0[h]
                    q64t, k64t = qk64[h]
                    s2 = spsum.tile([P, 2 * QG], F32, tag="s", name="s")
                    # two k-tiles in disjoint PE row-groups (base 0 /
                    # base 64) -> the array runs them concurrently
                    nc.tensor.matmul(
                        s2[:, :QG],
                        k0t[0:DH, j0 * P:(j0 + 1) * P],
                        q0t[0:DH, gs],
                        start=True, stop=True,
                    )
                    nc.tensor.matmul(
                        s2[:, QG:],
                        k64t[DH:P, j1 * P:(j1 + 1) * P],
                        q64t[DH:P, gs],
                        start=True, stop=True,
                    )
                    p2 = ppool.tile([P, 2 * QG], F32R, tag="p", name="p")
                    nc.scalar.activation(p2[:], s2[:], EXP, scale=float(SCALE))
                    if pr >= npairs - 2:
                        nc.vector.tensor_mul(
                            p2[:], p2[:], m2[pr - (npairs - 2)][:]
                        )
                    nc.tensor.matmul(
                        yt[h][:], v1ap(h, j0), p2[:, :QG],
                        start=(pr == 0), stop=False,
                    )
                    nc.tensor.matmul(
                        yt[h][:], v1ap(h, j1), p2[:, QG:],
                        start=False, stop=(pr == npairs - 1),
                    )
                    # interleave next group's QKV as PE filler
                    while (fill and
                           next_fill <= (ui + 1) * nf_total // n_att):
                        gg, u = fill.pop(0)
                        emit_qkv_unit(gg, u)
                        next_fill += 1
            for h in range(HPC):
                # divide by the softmax denominator (row DH of yt): fast
                # reciprocal -> broadcast along partitions via a K=1 matmul
                # against a ones row -> multiply, then add the v bias.
                r_t = rrp.tile([1, QG], F32R, tag="r", name="r")
                # custom-DVE reciprocal_approx_fast corrupts when reading
                # PSUM: bounce the denominator through SBUF first.
                d_f = rrp.tile([1, QG], F32, tag="df", name="df")
                nc.vector.tensor_copy(d_f[:], yt[h][DH:DH + 1, :])
                r_f = rrp.tile([1, QG], F32, tag="rf", name="rf")
                nc.vector.reciprocal_approx_fast(r_f[:], d_f[:])
                nc.vector.tensor_copy(r_t[:], r_f[:])
                R_t = aux.tile([DH, QG], F32, tag="aux", name="Rb")
                nc.tensor.matmul(R_t[:], ones1[:], r_t[:],
                                 start=True, stop=True)
                Rs = rrp.tile([DH, QG], F32, tag="Rs", name="Rs")
                nc.vector.tensor_copy(Rs[:], R_t[:])
                dest = yt0[DH * h:DH * (h + 1), :] if h < 2 else yt1[:]
                nc.vector.tensor_mul(dest, yt[h][:DH, :], Rs[:])
                bvh = bv_sb[DH * h:DH * (h + 1), :] if h < 2 else bv2_sb[:]
                nc.vector.tensor_scalar_add(dest, dest, bvh)
            # output projection for this query group
            for cm in range(6):
                op = aux.tile([P, QG], F32, tag="aux", name="oo")
                nc.tensor.matmul(
                    op[:], wp0[:, cm * P:(cm + 1) * P], yt0[:],
                    start=True, stop=False,
                )
                nc.tensor.matmul(
                    op[:], wp1[:, cm * P:(cm + 1) * P], yt1[:],
                    start=False, stop=True,
                )
                ob = osb.tile([P, QG], F32, tag="ob", name="ob")
                nc.vector.tensor_copy(ob[:], op[:])
                nc.sync.dma_start(
                    outT[cm * P:(cm + 1) * P, g * QG:(g + 1) * QG], ob[:]
                )
    nc.compile()
    return nc


_NC_CACHE = {}


def get_nc(t=T_FULL):
    if t not in _NC_CACHE:
        _NC_CACHE[t] = build_nc(t)
    return _NC_CACHE[t]


def make_in_maps(x, W_attn, b_attn, W_proj):
    x = np.ascontiguousarray(np.asarray(x, np.float32))
    W_attn = np.asarray(W_attn, np.float32)
    b_attn = np.asarray(b_attn, np.float32)
    W_proj = np.asarray(W_proj, np.float32)
    in_maps = []
    for c in range(NCORES):
        b = c // 4
        hs = [3 * (c % 4) + i for i in range(HPC)]
        cols = [W_attn[:, BLK[kind] * C + hs[lh] * DH:
                       BLK[kind] * C + (hs[lh] + 1) * DH]
                for kind, lh in QK_ORDER]
        wqk = np.ascontiguousarray(np.concatenate(cols, axis=1))
        bqk = np.concatenate(
            [b_attn[BLK[kind] * C + hs[lh] * DH:BLK[kind] * C + (hs[lh] + 1) * DH]
             for kind, lh in QK_ORDER]
        ).reshape(NQK, 1)
        wvp = np.zeros((C, NVP), np.float32)
        for lh in range(HPC):
            wvp[:, 65 * lh:65 * lh + DH] = \
                W_attn[:, 2 * C + hs[lh] * DH:2 * C + (hs[lh] + 1) * DH]
        bv = np.concatenate(
            [b_attn[2 * C + h * DH:2 * C + (h + 1) * DH] for h in hs]
        ).reshape(HPC * DH, 1)
        wp = np.ascontiguousarray(
            np.concatenate([W_proj[h * DH:(h + 1) * DH, :] for h in hs], axis=0)
        )
        xTc = np.ascontiguousarray(x[b].T)
        in_maps.append({
            "xT": xTc,
            "wqk": wqk,
            "bqk": np.ascontiguousarray(bqk),
            "wvp": wvp,
            "bv": np.ascontiguousarray(bv),
            "wp": wp,
        })
    return in_maps


def unshard(per_core_outT, b_proj):
    t = per_core_outT[0].shape[1]
    out = np.zeros((B, t, C), np.float32)
    for c in range(NCORES):
        out[c // 4] += per_core_outT[c].T
    out += np.asarray(b_proj, np.float32)[None, None, :]
    return out


def kernel(x, W_attn, b_attn, W_proj, b_proj, **run_kwargs):
    nc = get_nc(T_FULL)
    in_maps = make_in_maps(x, W_attn, b_attn, W_proj)
    res = None
    last_err = None
    for attempt in range(3):
        try:
            res = run_bass_kernel_spmd(nc, in_maps,
                                       core_ids=list(range(NCORES)),
                                       **run_kwargs)
            break
        except Exception as e:  # transient NRT_EXEC_UNIT_UNRECOVERABLE etc.
            last_err = e
    if res is None:
        raise last_err
    outs = [res.results[c]["outT"] for c in range(NCORES)]
    out = unshard(outs, b_proj)
    return out



# revision 15
# speedup vs baseline: 1.2009x; 1.2009x over previous
"""Causal self-attention (B=2, T=4096, C=768, H=12) on 8 TRN2 NeuronCores.

Sharding: core c -> batch c//4, heads 3*(c%4) .. 3*(c%4)+2.  Each core is
fully independent (no collectives): it computes qkv for its 3 heads from
x[b], runs causal flash attention, and produces the partial output
projection outT = (Y_heads @ W_proj[rows]).T of shape [C, T].  The host
sums the 4 per-batch partials, transposes, and adds b_proj.

Engine-balanced design (ACT exp ~235us is the floor; everything else
hides under it):
  - PE: S^T pairs in disjoint row-groups (base-0/base-64) AND PV split
    into two 64-contraction halves (rows 0-63 -> ytA bank, 64-127 -> ytB
    bank) that also run concurrently.  Head-major loop so only 2 yt PSUM
    banks live; next-group QKV and prev-group proj interleave as filler.
  - ACT: only the 216 exp activations.
  - DVE: qkT bias-adds, vnat drains (+v-bias fold), ytA+ytB merge,
    reciprocal, normalize-mul, proj drains.
  - Pool: causal masks via in-place affine_select on p2, reciprocal
    partition-broadcast.
  - DMA: half-swapped qkT duplicates via SBUF->SBUF copies.
"""

import os
import sys

import numpy as np

for _p in ("/opt/trn_rl_repo", "/root/.axon_site/_ro/trn_rl_repo"):
    if os.path.isdir(_p) and _p not in sys.path:
        sys.path.insert(0, _p)

from contextlib import ExitStack

import concourse.bacc as bacc
import concourse.bass as bass
import concourse.mybir as mybir
import concourse.tile as tile
from concourse.bass_utils import run_bass_kernel_spmd

F32 = mybir.dt.float32
F32R = mybir.dt.float32r
EXP = mybir.ActivationFunctionType.Exp
IS_GE = mybir.AluOpType.is_ge

B, T_FULL, C = 2, 4096, 768
H, DH = 12, 64
HPC = 3                      # heads per core
NCORES = 8
P = 128
QG = 512                     # query-group span (free dim of S^T tiles)
KT = 128                     # key tile (partition dim of S^T tiles)
NQK = 6 * DH                 # 384 rows of qkT (q,k for 3 heads)
NVP = 256                    # padded v matmul width: v0|1|v1|1|v2|1|zeros
VW = 3 * 65                  # = 195 columns of vnat actually kept
SCALE = 1.0 / np.sqrt(DH)

# qkT row layout as (block, local_head) pairs of 64 rows each
QK_ORDER = [("q", 0), ("q", 1), ("k", 0), ("k", 1), ("q", 2), ("k", 2)]
BLK = {"q": 0, "k": 1, "v": 2}


def r32(ap):
    return ap.bitcast(F32R)


def build_nc(t=T_FULL, debug=False):
    ng = t // QG             # query groups
    nc = bacc.Bacc(None, target_bir_lowering=False)
    dbg = {}
    if debug:
        for name, shape in [("d_qkt0", [P, t]), ("d_qkd0", [P, t]),
                            ("d_vn0", [P, VW]), ("d_bvt", [P, NVP]),
                            ("d_p2", [P, 2 * QG]), ("d_yts", [DH + 1, QG]),
                            ("d_rb", [DH, QG])]:
            dbg[name] = nc.declare_dram_parameter(name, shape, F32,
                                                  isOutput=True)
        nc._dbg_tensors = list(dbg)
    xT = nc.declare_dram_parameter("xT", [C, t], F32, isOutput=False)
    wqk = nc.declare_dram_parameter("wqk", [C, NQK], F32, isOutput=False)
    bqk = nc.declare_dram_parameter("bqk", [NQK, 1], F32, isOutput=False)
    wvp = nc.declare_dram_parameter("wvp", [C, NVP], F32, isOutput=False)
    bvrow = nc.declare_dram_parameter("bvrow", [1, NVP], F32, isOutput=False)
    wp = nc.declare_dram_parameter("wp", [HPC * DH, C], F32, isOutput=False)
    outT = nc.declare_dram_parameter("outT", [C, t], F32, isOutput=True)

    with tile.TileContext(nc) as tc, ExitStack() as ctx:
        const = ctx.enter_context(tc.tile_pool(name="const", bufs=1))
        qkp = ctx.enter_context(tc.tile_pool(name="qk", bufs=1))
        vp = ctx.enter_context(tc.tile_pool(name="vn", bufs=1))
        xpool = ctx.enter_context(tc.tile_pool(name="xin", bufs=1))
        ppool = ctx.enter_context(tc.tile_pool(name="pp", bufs=2))
        ytsb = ctx.enter_context(tc.tile_pool(name="ytsb", bufs=2))
        ytssb = ctx.enter_context(tc.tile_pool(name="ytssb", bufs=2))
        rrp = ctx.enter_context(tc.tile_pool(name="rr", bufs=2))
        osb = ctx.enter_context(tc.tile_pool(name="osb", bufs=2))
        spsum = ctx.enter_context(tc.tile_pool(name="sps", bufs=2, space="PSUM"))
        ytps = ctx.enter_context(tc.tile_pool(name="ytps", bufs=1, space="PSUM"))
        aux = ctx.enter_context(tc.tile_pool(name="aux", bufs=2, space="PSUM"))

        # ---- constants -------------------------------------------------
        wqk_sb, wvp_sb = [], []
        for k in range(6):
            tl = const.tile([P, NQK], F32R, tag=f"wqk{k}", name=f"wqk{k}")
            nc.sync.dma_start(tl[:], r32(wqk[k * P:(k + 1) * P, :]))
            wqk_sb.append(tl)
            tv = const.tile([P, NVP], F32R, tag=f"wvp{k}", name=f"wvp{k}")
            nc.sync.dma_start(tv[:], r32(wvp[k * P:(k + 1) * P, :]))
            wvp_sb.append(tv)
        b_sb = []
        for m in range(3):
            tl = const.tile([P, 1], F32, tag=f"bq{m}", name=f"bq{m}")
            nc.sync.dma_start(tl[:], bqk[m * P:(m + 1) * P, :])
            b_sb.append(tl)
        # v-bias row (with 1.0 in the ones columns), broadcast to all
        # partitions once: vnat drain adds it, so PV emits (Y + bv*denom)
        # rows plus the raw denominator, and the normalize-divide yields
        # Y/denom + bv with no separate bias op.
        bvr_sb = const.tile([1, NVP], F32, tag="bvr", name="bvr")
        nc.sync.dma_start(bvr_sb[:], bvrow[:, :])
        bv_tile = const.tile([P, NVP], F32, tag="bvt", name="bvt")
        nc.gpsimd.partition_broadcast(bv_tile[:], bvr_sb[:])
        wp0 = const.tile([P, C], F32R, tag="wp0", name="wp0")
        wp1 = const.tile([DH, C], F32R, tag="wp1", name="wp1")
        nc.sync.dma_start(wp0[:], r32(wp[0:P, :]))
        nc.sync.dma_start(wp1[:], r32(wp[P:P + DH, :]))

        # ---- persistent qkT / v storage --------------------------------
        # qkT row layout: A=[q0|q1] B=[k0|k1] C=[q2|k2], plus half-swapped
        # duplicates A'=[q1|q0] B'=[k1|k0] C'=[k2|q2] so every head has its
        # (q, k) pair available at base partition 0 AND base partition 64.
        qkt = [qkp.tile([P, t], F32R, tag=f"qkt{i}", name=f"qkt{i}")
               for i in range(3)]          # A, B, C
        qkd = [qkp.tile([P, t], F32R, tag=f"qkd{i}", name=f"qkd{i}")
               for i in range(3)]          # A', B', C'
        A, Bt, Ct = qkt
        Ad, Bd, Cd = qkd
        qk0 = [(A, Bt), (Ad, Bd), (Ct, Cd)]      # base-0 (q_tile, k_tile)
        qk64 = [(Ad, Bd), (A, Bt), (Cd, Ct)]     # base-64 (q_tile, k_tile)
        # v in natural orientation: per 128-row t-tile, cols h*65..h*65+64
        # hold [v_h | ones]
        vnat = [vp.tile([P, VW], F32R, tag=f"vn{j}", name=f"vn{j}")
                for j in range(t // P)]

        def v_lo(h, j):
            return vnat[j][0:DH, 65 * h:65 * h + DH + 1]

        def v_hi(h, j):
            return vnat[j][DH:P, 65 * h:65 * h + DH + 1]

        # ---- per-group QKV emission (used as PE filler) ----------------
        def emit_qkv_unit(g, u):
            """Emit filler unit u (0..7) of query group g's QKV work."""
            gs = slice(g * QG, (g + 1) * QG)
            if u == 0:
                xk = []
                for k in range(6):
                    tl = xpool.tile([P, QG], F32R, tag=f"x{k}", name=f"x{k}")
                    nc.sync.dma_start(tl[:], r32(xT[k * P:(k + 1) * P, gs]))
                    xk.append(tl)
                xtiles[g % 2] = xk
                return
            xk = xtiles[g % 2]
            if u <= 3:
                m = u - 1           # qkT m-tile (A, B, C)
                ps = aux.tile([P, QG], F32, tag="aux", name="qkps")
                for k in range(6):
                    nc.tensor.matmul(ps[:], wqk_sb[k][:, m * P:(m + 1) * P],
                                     xk[k][:], start=(k == 0), stop=(k == 5))
                nc.vector.tensor_scalar_add(qkt[m][:, gs], ps[:], b_sb[m][:])
                # half-swapped duplicate via SBUF->SBUF DMA (frees ACT)
                nc.sync.dma_start(qkd[m][DH:, gs], qkt[m][:DH, gs])
                nc.sync.dma_start(qkd[m][:DH, gs], qkt[m][DH:, gs])
            else:
                ti = u - 4          # v t-tile within the group (0..3)
                j = 4 * g + ti
                ps = aux.tile([P, NVP], F32, tag="aux", name="vnps")
                for k in range(6):
                    nc.tensor.matmul(ps[:], xk[k][:, ti * P:(ti + 1) * P],
                                     wvp_sb[k][:], start=(k == 0), stop=(k == 5))
                nc.vector.tensor_add(vnat[j][:], ps[:, :VW], bv_tile[:, :VW])

        N_QKV_UNITS = 8  # 1 dma + 3 qk + 4 v

        def emit_proj_unit(g, cm, y0, y1):
            """Output projection for column-tile cm of group g."""
            op = aux.tile([P, QG], F32, tag="aux", name="oo")
            nc.tensor.matmul(
                op[:], wp0[:, cm * P:(cm + 1) * P], y0[:],
                start=True, stop=False,
            )
            nc.tensor.matmul(
                op[:], wp1[:, cm * P:(cm + 1) * P], y1[:],
                start=False, stop=True,
            )
            ob = osb.tile([P, QG], F32, tag="ob", name="ob")
            nc.vector.tensor_copy(ob[:], op[:])
            nc.sync.dma_start(
                outT[cm * P:(cm + 1) * P, g * QG:(g + 1) * QG], ob[:]
            )

        xtiles = [None, None]
        # prologue: group 0's QKV
        for u in range(N_QKV_UNITS):
            emit_qkv_unit(0, u)

        # ---- fused attention + filler (QKV one group ahead, proj one
        # ---- group behind) --------------------------------------------
        prev_y = None            # (yt0, yt1) of the previous group
        for g in range(ng):
            gs = slice(g * QG, (g + 1) * QG)
            npairs = 2 * (g + 1)
            yt0 = ytsb.tile([P, QG], F32R, tag="yt0", name="yt0")
            yt1 = ytsb.tile([DH, QG], F32R, tag="yt1", name="yt1")
            # filler: next group's QKV interleaved with prev group's proj
            fill = []
            if g + 1 < ng:
                fill = [("qkv", g + 1, u) for u in range(N_QKV_UNITS)]
            if prev_y is not None:
                pf = [("proj", g - 1, cm) for cm in range(6)]
                # weave proj after the first few QKV units (prev group's
                # normalization output must be ready)
                fill = fill[:3] + [x for pair in zip(fill[3:], pf)
                                   for x in pair] + pf[len(fill[3:]):]
            nf_total = len(fill)
            n_att = npairs * HPC
            next_fill = 0
            ui = 0
            for h in range(HPC):
                q0t, k0t = qk0[h]
                q64t, k64t = qk64[h]
                ytA = ytps.tile([DH + 1, QG], F32, tag="ytA", name="ytA")
                ytB = ytps.tile([DH + 1, QG], F32, tag="ytB", name="ytB")
                for pr in range(npairs):
                    j0, j1 = 2 * pr, 2 * pr + 1
                    s2 = spsum.tile([P, 2 * QG], F32, tag="s", name="s")
                    # two k-tiles in disjoint PE row-groups (base 0 /
                    # base 64) -> the array runs them concurrently
                    nc.tensor.matmul(
                        s2[:, :QG],
                        k0t[0:DH, j0 * P:(j0 + 1) * P],
                        q0t[0:DH, gs],
                        start=True, stop=True,
                    )
                    nc.tensor.matmul(
                        s2[:, QG:],
                        k64t[DH:P, j1 * P:(j1 + 1) * P],
                        q64t[DH:P, gs],
                        start=True, stop=True,
                    )
                    p2 = ppool.tile([P, 2 * QG], F32R, tag="p", name="p")
                    nc.scalar.activation(p2[:], s2[:], EXP, scale=float(SCALE))
                    if pr >= npairs - 2:
                        # causal mask for the 2 diagonal k-tiles, in-place
                        # on the Pool engine: keep col q of row k of half
                        # j iff (g*512 + q) - (j*128 + k) >= 0.
                        nc.gpsimd.affine_select(
                            out=p2[:],
                            in_=p2[:],
                            compare_op=IS_GE,
                            fill=0.0,
                            base=g * QG - j0 * KT,
                            pattern=[[-KT, 2], [1, QG]],
                            channel_multiplier=-1,
                        )
                    if debug and g == 0 and h == 0 and pr == npairs - 1:
                        nc.sync.dma_start(dbg["d_p2"][:, :],
                                          p2[:].bitcast(F32))
                    # split PV: rows 0-63 -> ytA, rows 64-127 -> ytB in
                    # disjoint row-groups (concurrent pairs per k-tile)
                    first, last = (pr == 0), (pr == npairs - 1)
                    nc.tensor.matmul(ytA[:], v_lo(h, j0), p2[0:DH, :QG],
                                     start=first, stop=False)
                    nc.tensor.matmul(ytB[:], v_hi(h, j0), p2[DH:P, :QG],
                                     start=first, stop=False)
                    nc.tensor.matmul(ytA[:], v_lo(h, j1), p2[0:DH, QG:],
                                     start=False, stop=last)
                    nc.tensor.matmul(ytB[:], v_hi(h, j1), p2[DH:P, QG:],
                                     start=False, stop=last)
                    ui += 1
                    while fill and next_fill <= ui * nf_total // n_att:
                        kind, gg, u = fill.pop(0)
                        if kind == "qkv":
                            emit_qkv_unit(gg, u)
                        else:
                            emit_proj_unit(gg, u, *prev_y)
                        next_fill += 1
                # head-end normalization: merge the two PV halves, divide
                # rows 0-63 by the denominator row (row 64), all off-PE.
                yts = ytssb.tile([DH + 1, QG], F32, tag="yts", name="yts")
                ybs = ytssb.tile([DH + 1, QG], F32, tag="ybs", name="ybs")
                nc.vector.tensor_copy(ybs[:], ytB[:])
                nc.vector.tensor_add(yts[:], ytA[:], ybs[:])
                # reciprocal_approx_fast requires a partition-base-0 input
                # on hardware: bounce the denominator row (partition 64)
                # into its own [1, QG] tile first.
                d_f = rrp.tile([1, QG], F32, tag="df", name="df")
                nc.vector.tensor_copy(d_f[:], yts[DH:DH + 1, :])
                r_f = rrp.tile([1, QG], F32, tag="rf", name="rf")
                nc.vector.reciprocal_approx_fast(r_f[:], d_f[:])
                Rb = rrp.tile([DH, QG], F32, tag="Rb", name="Rb")
                nc.gpsimd.partition_broadcast(Rb[:], r_f[:])
                dest = yt0[DH * h:DH * (h + 1), :] if h < 2 else yt1[:]
                nc.vector.tensor_mul(dest, yts[:DH, :], Rb[:])
                if debug and g == 0 and h == 0:
                    nc.sync.dma_start(dbg["d_yts"][:, :], yts[:])
                    nc.sync.dma_start(dbg["d_rb"][:, :], Rb[:])
            while fill:
                kind, gg, u = fill.pop(0)
                if kind == "qkv":
                    emit_qkv_unit(gg, u)
                else:
                    emit_proj_unit(gg, u, *prev_y)
            prev_y = (yt0, yt1)
        # epilogue: last group's projection
        for cm in range(6):
            emit_proj_unit(ng - 1, cm, *prev_y)
        if debug:
            nc.sync.dma_start(dbg["d_qkt0"][:, :], qkt[0][:].bitcast(F32))
            nc.sync.dma_start(dbg["d_qkd0"][:, :], qkd[0][:].bitcast(F32))
            nc.sync.dma_start(dbg["d_vn0"][:, :], vnat[0][:].bitcast(F32))
            nc.sync.dma_start(dbg["d_bvt"][:, :], bv_tile[:])
    nc.compile()
    return nc


_NC_CACHE = {}


def get_nc(t=T_FULL):
    if t not in _NC_CACHE:
        _NC_CACHE[t] = build_nc(t)
    return _NC_CACHE[t]


def make_in_maps(x, W_attn, b_attn, W_proj):
    x = np.ascontiguousarray(np.asarray(x, np.float32))
    W_attn = np.asarray(W_attn, np.float32)
    b_attn = np.asarray(b_attn, np.float32)
    W_proj = np.asarray(W_proj, np.float32)
    in_maps = []
    for c in range(NCORES):
        b = c // 4
        hs = [3 * (c % 4) + i for i in range(HPC)]
        cols = [W_attn[:, BLK[kind] * C + hs[lh] * DH:
                       BLK[kind] * C + (hs[lh] + 1) * DH]
                for kind, lh in QK_ORDER]
        wqk = np.ascontiguousarray(np.concatenate(cols, axis=1))
        bqk = np.concatenate(
            [b_attn[BLK[kind] * C + hs[lh] * DH:BLK[kind] * C + (hs[lh] + 1) * DH]
             for kind, lh in QK_ORDER]
        ).reshape(NQK, 1)
        wvp = np.zeros((C, NVP), np.float32)
        bvrow = np.zeros((1, NVP), np.float32)
        for lh in range(HPC):
            wvp[:, 65 * lh:65 * lh + DH] = \
                W_attn[:, 2 * C + hs[lh] * DH:2 * C + (hs[lh] + 1) * DH]
            bvrow[0, 65 * lh:65 * lh + DH] = \
                b_attn[2 * C + hs[lh] * DH:2 * C + (hs[lh] + 1) * DH]
            bvrow[0, 65 * lh + DH] = 1.0
        wp = np.ascontiguousarray(
            np.concatenate([W_proj[h * DH:(h + 1) * DH, :] for h in hs], axis=0)
        )
        xTc = np.ascontiguousarray(x[b].T)
        in_maps.append({
            "xT": xTc,
            "wqk": wqk,
            "bqk": np.ascontiguousarray(bqk),
            "wvp": wvp,
            "bvrow": bvrow,
            "wp": wp,
        })
    return in_maps


def unshard(per_core_outT, b_proj):
    t = per_core_outT[0].shape[1]
    out = np.zeros((B, t, C), np.float32)
    for c in range(NCORES):
        out[c // 4] += per_core_outT[c].T
    out += np.asarray(b_proj, np.float32)[None, None, :]
    return out


def kernel(x, W_attn, b_attn, W_proj, b_proj, **run_kwargs):
    nc = get_nc(T_FULL)
    in_maps = make_in_maps(x, W_attn, b_attn, W_proj)
    res = None
    last_err = None
    for attempt in range(3):
        try:
            res = run_bass_kernel_spmd(nc, in_maps,
                                       core_ids=list(range(NCORES)),
                                       **run_kwargs)
            break
        except Exception as e:  # transient NRT_EXEC_UNIT_UNRECOVERABLE etc.
            last_err = e
    if res is None:
        raise last_err
    outs = [res.results[c]["outT"] for c in range(NCORES)]
    out = unshard(outs, b_proj)
    return out


# revision 20
# speedup vs baseline: 1.3737x; 1.1439x over previous
"""Causal self-attention (B=2, T=4096, C=768, H=12) on 8 TRN2 NeuronCores.

Sharding: core c -> batch c//4, heads 3*(c%4) .. 3*(c%4)+2.  Each core is
fully independent (no collectives): it computes qkv for its 3 heads from
x[b], runs causal flash attention, and produces the partial output
projection outT = (Y_heads @ W_proj[rows]).T of shape [C, T].  The host
sums the 4 per-batch partials, transposes, and adds b_proj.

Engine-balanced design (ACT exp ~235us is the floor; everything else
hides under it):
  - PE: S^T pairs in disjoint row-groups (base-0/base-64) AND PV split
    into two 64-contraction halves (rows 0-63 -> ytA bank, 64-127 -> ytB
    bank) that also run concurrently.  Head-major loop so only 2 yt PSUM
    banks live; next-group QKV and prev-group proj interleave as filler.
  - ACT: only the 216 exp activations.
  - DVE: qkT bias-adds, vnat drains (+v-bias fold), ytA+ytB merge,
    reciprocal, normalize-mul, proj drains.
  - Pool: causal masks via in-place affine_select on p2, reciprocal
    partition-broadcast.
  - DMA: half-swapped qkT duplicates via SBUF->SBUF copies.
"""

import os
import sys

import numpy as np

for _p in ("/opt/trn_rl_repo", "/root/.axon_site/_ro/trn_rl_repo"):
    if os.path.isdir(_p) and _p not in sys.path:
        sys.path.insert(0, _p)

from contextlib import ExitStack

import concourse.bacc as bacc
import concourse.bass as bass
import concourse.mybir as mybir
import concourse.tile as tile
from concourse.bass_utils import run_bass_kernel_spmd

F32 = mybir.dt.float32
F32R = mybir.dt.float32r
EXP = mybir.ActivationFunctionType.Exp
IS_GE = mybir.AluOpType.is_ge

B, T_FULL, C = 2, 4096, 768
H, DH = 12, 64
HPC = 3                      # heads per core
NCORES = 8
P = 128
QG = 512                     # query-group span (free dim of S^T tiles)
KT = 128                     # key tile (partition dim of S^T tiles)
NQK = 6 * DH                 # 384 rows of qkT (q,k for 3 heads)
NVP = 256                    # padded v matmul width: v0|1|v1|1|v2|1|zeros
VW = 3 * 65                  # = 195 columns of vnat actually kept
SCALE = 1.0 / np.sqrt(DH)

# qkT row layout as (block, local_head) pairs of 64 rows each
QK_ORDER = [("q", 0), ("q", 1), ("k", 0), ("k", 1), ("q", 2), ("k", 2)]
BLK = {"q": 0, "k": 1, "v": 2}


def r32(ap):
    return ap.bitcast(F32R)


def build_nc(t=T_FULL, debug=False):
    ng = t // QG             # query groups
    nc = bacc.Bacc(None, target_bir_lowering=False)
    dbg = {}
    if debug:
        for name, shape in [("d_qkt0", [P, t]), ("d_qkd0", [P, t]),
                            ("d_vn0", [P, VW]), ("d_bvt", [P, NVP]),
                            ("d_p2", [P, 2 * QG]), ("d_yts", [DH + 1, QG]),
                            ("d_rb", [DH, QG])]:
            dbg[name] = nc.declare_dram_parameter(name, shape, F32,
                                                  isOutput=True)
        nc._dbg_tensors = list(dbg)
    xT = nc.declare_dram_parameter("xT", [C, t], F32, isOutput=False)
    wqk = nc.declare_dram_parameter("wqk", [C, NQK], F32, isOutput=False)
    bqk = nc.declare_dram_parameter("bqk", [NQK, 1], F32, isOutput=False)
    wvp = nc.declare_dram_parameter("wvp", [C, NVP], F32, isOutput=False)
    bvrow = nc.declare_dram_parameter("bvrow", [1, NVP], F32, isOutput=False)
    wp = nc.declare_dram_parameter("wp", [HPC * DH, C], F32, isOutput=False)
    outT = nc.declare_dram_parameter("outT", [C, t], F32, isOutput=True)

    with tile.TileContext(nc) as tc, ExitStack() as ctx:
        const = ctx.enter_context(tc.tile_pool(name="const", bufs=1))
        qkp = ctx.enter_context(tc.tile_pool(name="qk", bufs=1))
        vp = ctx.enter_context(tc.tile_pool(name="vn", bufs=1))
        xpool = ctx.enter_context(tc.tile_pool(name="xin", bufs=1))
        ppool = ctx.enter_context(tc.tile_pool(name="pp", bufs=3))
        ytsb = ctx.enter_context(tc.tile_pool(name="ytsb", bufs=2))
        ytssb = ctx.enter_context(tc.tile_pool(name="ytssb", bufs=2))
        rrp = ctx.enter_context(tc.tile_pool(name="rr", bufs=2))
        osb = ctx.enter_context(tc.tile_pool(name="osb", bufs=2))
        spsum = ctx.enter_context(tc.tile_pool(name="sps", bufs=2, space="PSUM"))
        ytps = ctx.enter_context(tc.tile_pool(name="ytps", bufs=1, space="PSUM"))
        aux = ctx.enter_context(tc.tile_pool(name="aux", bufs=2, space="PSUM"))

        # ---- constants -------------------------------------------------
        # DMA order is the critical-path order: the k-th qk matmul of the
        # prologue only needs wqk[k] and x0[k], so interleave them and
        # defer everything not needed until later in the stream.
        wqk_sb, wvp_sb, x0_sb = [], [], []
        b_sb = []
        bvr_sb = const.tile([1, NVP], F32, tag="bvr", name="bvr")
        for k in range(6):
            tl = const.tile([P, NQK], F32R, tag=f"wqk{k}", name=f"wqk{k}")
            nc.sync.dma_start(tl[:], r32(wqk[k * P:(k + 1) * P, :]))
            wqk_sb.append(tl)
            tx = xpool.tile([P, QG], F32R, tag=f"x{k}", name=f"x{k}")
            nc.sync.dma_start(tx[:], r32(xT[k * P:(k + 1) * P, 0:QG]))
            x0_sb.append(tx)
            if k == 0:
                for m in range(3):
                    tb = const.tile([P, 1], F32, tag=f"bq{m}", name=f"bq{m}")
                    nc.sync.dma_start(tb[:], bqk[m * P:(m + 1) * P, :])
                    b_sb.append(tb)
                nc.sync.dma_start(bvr_sb[:], bvrow[:, :])
        for k in range(6):
            tv = const.tile([P, NVP], F32R, tag=f"wvp{k}", name=f"wvp{k}")
            nc.sync.dma_start(tv[:], r32(wvp[k * P:(k + 1) * P, :]))
            wvp_sb.append(tv)
        # v-bias row (with 1.0 in the ones columns), broadcast to all
        # partitions once: vnat drain adds it, so PV emits (Y + bv*denom)
        # rows plus the raw denominator, and the normalize-divide yields
        # Y/denom + bv with no separate bias op.
        bv_tile = const.tile([P, NVP], F32, tag="bvt", name="bvt")
        nc.gpsimd.partition_broadcast(bv_tile[:], bvr_sb[:])
        wp0 = const.tile([P, C], F32R, tag="wp0", name="wp0")
        wp1 = const.tile([DH, C], F32R, tag="wp1", name="wp1")
        nc.sync.dma_start(wp0[:], r32(wp[0:P, :]))
        nc.sync.dma_start(wp1[:], r32(wp[P:P + DH, :]))

        # ---- persistent qkT / v storage --------------------------------
        # qkT row layout: A=[q0|q1] B=[k0|k1] C=[q2|k2], plus half-swapped
        # duplicates A'=[q1|q0] B'=[k1|k0] C'=[k2|q2] so every head has its
        # (q, k) pair available at base partition 0 AND base partition 64.
        qkt = [qkp.tile([P, t], F32R, tag=f"qkt{i}", name=f"qkt{i}")
               for i in range(3)]          # A, B, C
        qkd = [qkp.tile([P, t], F32R, tag=f"qkd{i}", name=f"qkd{i}")
               for i in range(3)]          # A', B', C'
        A, Bt, Ct = qkt
        Ad, Bd, Cd = qkd
        qk0 = [(A, Bt), (Ad, Bd), (Ct, Cd)]      # base-0 (q_tile, k_tile)
        qk64 = [(Ad, Bd), (A, Bt), (Cd, Ct)]     # base-64 (q_tile, k_tile)
        # v in natural orientation: per 128-row t-tile, cols h*65..h*65+64
        # hold [v_h | ones]
        vnat = [vp.tile([P, VW], F32R, tag=f"vn{j}", name=f"vn{j}")
                for j in range(t // P)]

        def v_lo(h, j):
            return vnat[j][0:DH, 65 * h:65 * h + DH + 1]

        def v_hi(h, j):
            return vnat[j][DH:P, 65 * h:65 * h + DH + 1]

        # ---- per-group QKV emission (used as PE filler) ----------------
        def emit_qkv_unit(g, u):
            """Emit filler unit u (0..7) of query group g's QKV work."""
            gs = slice(g * QG, (g + 1) * QG)
            if u == 0:
                if g == 0:
                    xtiles[0] = x0_sb       # preloaded with the constants
                    return
                xk = []
                for k in range(6):
                    tl = xpool.tile([P, QG], F32R, tag=f"x{k}", name=f"x{k}")
                    nc.sync.dma_start(tl[:], r32(xT[k * P:(k + 1) * P, gs]))
                    xk.append(tl)
                xtiles[g % 2] = xk
                return
            xk = xtiles[g % 2]
            if u <= 3:
                m = u - 1           # qkT m-tile (A, B, C)
                ps = aux.tile([P, QG], F32, tag="aux", name="qkps")
                for k in range(6):
                    nc.tensor.matmul(ps[:], wqk_sb[k][:, m * P:(m + 1) * P],
                                     xk[k][:], start=(k == 0), stop=(k == 5))
                nc.vector.tensor_scalar_add(qkt[m][:, gs], ps[:], b_sb[m][:])
                # half-swapped duplicate via SBUF->SBUF DMA (frees ACT)
                nc.sync.dma_start(qkd[m][DH:, gs], qkt[m][:DH, gs])
                nc.sync.dma_start(qkd[m][:DH, gs], qkt[m][DH:, gs])
            else:
                ti = u - 4          # v t-tile within the group (0..3)
                j = 4 * g + ti
                ps = aux.tile([P, NVP], F32, tag="aux", name="vnps")
                for k in range(6):
                    nc.tensor.matmul(ps[:], xk[k][:, ti * P:(ti + 1) * P],
                                     wvp_sb[k][:], start=(k == 0), stop=(k == 5))
                nc.vector.tensor_add(vnat[j][:], ps[:, :VW], bv_tile[:, :VW])

        N_QKV_UNITS = 8  # 1 dma + 3 qk + 4 v

        def emit_proj_unit(g, cm, y0, y1):
            """Output projection for column-tile cm of group g."""
            op = aux.tile([P, QG], F32, tag="aux", name="oo")
            nc.tensor.matmul(
                op[:], wp0[:, cm * P:(cm + 1) * P], y0[:],
                start=True, stop=False,
            )
            nc.tensor.matmul(
                op[:], wp1[:, cm * P:(cm + 1) * P], y1[:],
                start=False, stop=True,
            )
            ob = osb.tile([P, QG], F32, tag="ob", name="ob")
            nc.vector.tensor_copy(ob[:], op[:])
            nc.sync.dma_start(
                outT[cm * P:(cm + 1) * P, g * QG:(g + 1) * QG], ob[:]
            )

        def emit_S(g, h, pr, npairs):
            """S^T pair + exp (+mask); returns the pending-PV job."""
            gs = slice(g * QG, (g + 1) * QG)
            q0t, k0t = qk0[h]
            q64t, k64t = qk64[h]
            j0, j1 = 2 * pr, 2 * pr + 1
            s2 = spsum.tile([P, 2 * QG], F32, tag="s", name="s")
            # two k-tiles in disjoint PE row-groups (base 0 / base 64) ->
            # the array runs them concurrently
            nc.tensor.matmul(
                s2[:, :QG],
                k0t[0:DH, j0 * P:(j0 + 1) * P],
                q0t[0:DH, gs],
                start=True, stop=True,
            )
            nc.tensor.matmul(
                s2[:, QG:],
                k64t[DH:P, j1 * P:(j1 + 1) * P],
                q64t[DH:P, gs],
                start=True, stop=True,
            )
            p2 = ppool.tile([P, 2 * QG], F32R, tag="p", name="p")
            nc.scalar.activation(p2[:], s2[:], EXP, scale=float(SCALE))
            if pr >= npairs - 2:
                # causal mask for the 2 diagonal k-tiles, in-place on the
                # Pool engine: keep col q of row k of half j iff
                # (g*512 + q) - (j*128 + k) >= 0.
                nc.gpsimd.affine_select(
                    out=p2[:],
                    in_=p2[:],
                    compare_op=IS_GE,
                    fill=0.0,
                    base=g * QG - j0 * KT,
                    pattern=[[-KT, 2], [1, QG]],
                    channel_multiplier=-1,
                )
            if debug and g == 0 and h == 0 and pr == npairs - 1:
                nc.sync.dma_start(dbg["d_p2"][:, :], p2[:].bitcast(F32))
            return (g, h, pr, npairs, p2)

        cur_yt = [None]          # live (ytA, ytB) PSUM accumulators

        def emit_PV(job, yt0, yt1):
            """PV quad for a pending job; norm chain at head end."""
            g, h, pr, npairs, p2 = job
            first, last = (pr == 0), (pr == npairs - 1)
            j0, j1 = 2 * pr, 2 * pr + 1
            if first:
                cur_yt[0] = (
                    ytps.tile([DH + 1, QG], F32, tag="ytA", name="ytA"),
                    ytps.tile([DH + 1, QG], F32, tag="ytB", name="ytB"),
                )
            ytA, ytB = cur_yt[0]
            # split PV: rows 0-63 -> ytA, rows 64-127 -> ytB in disjoint
            # row-groups (concurrent pairs per k-tile)
            nc.tensor.matmul(ytA[:], v_lo(h, j0), p2[0:DH, :QG],
                             start=first, stop=False)
            nc.tensor.matmul(ytB[:], v_hi(h, j0), p2[DH:P, :QG],
                             start=first, stop=False)
            nc.tensor.matmul(ytA[:], v_lo(h, j1), p2[0:DH, QG:],
                             start=False, stop=last)
            nc.tensor.matmul(ytB[:], v_hi(h, j1), p2[DH:P, QG:],
                             start=False, stop=last)
            if not last:
                return
            # head-end normalization: merge the two PV halves, divide
            # rows 0-63 by the denominator row (row 64), all off-PE.
            yts = ytssb.tile([DH + 1, QG], F32, tag="yts", name="yts")
            ybs = ytssb.tile([DH + 1, QG], F32, tag="ybs", name="ybs")
            nc.vector.tensor_copy(ybs[:], ytB[:])
            nc.vector.tensor_add(yts[:], ytA[:], ybs[:])
            # reciprocal_approx_fast requires a partition-base-0 input on
            # hardware: bounce the denominator row (partition 64) into its
            # own [1, QG] tile first.
            d_f = rrp.tile([1, QG], F32, tag="df", name="df")
            nc.vector.tensor_copy(d_f[:], yts[DH:DH + 1, :])
            r_f = rrp.tile([1, QG], F32, tag="rf", name="rf")
            nc.vector.reciprocal_approx_fast(r_f[:], d_f[:])
            Rb = rrp.tile([DH, QG], F32, tag="Rb", name="Rb")
            nc.gpsimd.partition_broadcast(Rb[:], r_f[:])
            dest = yt0[DH * h:DH * (h + 1), :] if h < 2 else yt1[:]
            nc.vector.tensor_mul(dest, yts[:DH, :], Rb[:])
            if debug and g == 0 and h == 0:
                nc.sync.dma_start(dbg["d_yts"][:, :], yts[:])
                nc.sync.dma_start(dbg["d_rb"][:, :], Rb[:])

        xtiles = [None, None]
        # prologue: group 0's QKV
        for u in range(N_QKV_UNITS):
            emit_qkv_unit(0, u)

        # ---- fused attention, software-pipelined: PV trails S by DEPTH
        # ---- pair-slots (continuously across heads and groups) so the
        # ---- exp+mask latency never stalls the PE.  QKV of group g+1 and
        # ---- proj of group g-1 weave in as PE filler.
        DEPTH = 2
        pend = []                # jobs with S emitted, PV outstanding
        yts_of = {}              # group -> (yt0, yt1)
        prev_y = None
        for g in range(ng):
            npairs = 2 * (g + 1)
            yt0 = ytsb.tile([P, QG], F32R, tag="yt0", name="yt0")
            yt1 = ytsb.tile([DH, QG], F32R, tag="yt1", name="yt1")
            yts_of[g] = (yt0, yt1)
            # filler: next group's QKV interleaved with prev group's proj
            fill = []
            if g + 1 < ng:
                fill = [("qkv", g + 1, u) for u in range(N_QKV_UNITS)]
            if prev_y is not None:
                pf = [("proj", g - 1, cm) for cm in range(6)]
                # weave proj after the first few QKV units (prev group's
                # normalization output must be ready)
                fill = fill[:3] + [x for pair in zip(fill[3:], pf)
                                   for x in pair] + pf[len(fill[3:]):]
            nf_total = len(fill)
            n_att = npairs * HPC
            next_fill = 0
            ui = 0
            for h in range(HPC):
                for pr in range(npairs):
                    pend.append(emit_S(g, h, pr, npairs))
                    if len(pend) > DEPTH:
                        jb = pend.pop(0)
                        emit_PV(jb, *yts_of[jb[0]])
                    ui += 1
                    while fill and next_fill <= ui * nf_total // n_att:
                        kind, gg, u = fill[0]
                        if kind == "proj" and any(j[0] < g for j in pend):
                            # prev group's final norm not yet emitted
                            break
                        fill.pop(0)
                        if kind == "qkv":
                            emit_qkv_unit(gg, u)
                        else:
                            emit_proj_unit(gg, u, *prev_y)
                        next_fill += 1
            while fill:
                kind, gg, u = fill.pop(0)
                if kind == "qkv":
                    emit_qkv_unit(gg, u)
                else:
                    emit_proj_unit(gg, u, *prev_y)
            prev_y = (yt0, yt1)
        # epilogue: drain the pipeline, then the last group's projection
        while pend:
            jb = pend.pop(0)
            emit_PV(jb, *yts_of[jb[0]])
        for cm in range(6):
            emit_proj_unit(ng - 1, cm, *prev_y)
        if debug:
            nc.sync.dma_start(dbg["d_qkt0"][:, :], qkt[0][:].bitcast(F32))
            nc.sync.dma_start(dbg["d_qkd0"][:, :], qkd[0][:].bitcast(F32))
            nc.sync.dma_start(dbg["d_vn0"][:, :], vnat[0][:].bitcast(F32))
            nc.sync.dma_start(dbg["d_bvt"][:, :], bv_tile[:])
    nc.compile()
    return nc


_NC_CACHE = {}


def get_nc(t=T_FULL):
    if t not in _NC_CACHE:
        _NC_CACHE[t] = build_nc(t)
    return _NC_CACHE[t]


def make_in_maps(x, W_attn, b_attn, W_proj):
    x = np.ascontiguousarray(np.asarray(x, np.float32))
    W_attn = np.asarray(W_attn, np.float32)
    b_attn = np.asarray(b_attn, np.float32)
    W_proj = np.asarray(W_proj, np.float32)
    in_maps = []
    for c in range(NCORES):
        b = c // 4
        hs = [3 * (c % 4) + i for i in range(HPC)]
        cols = [W_attn[:, BLK[kind] * C + hs[lh] * DH:
                       BLK[kind] * C + (hs[lh] + 1) * DH]
                for kind, lh in QK_ORDER]
        wqk = np.ascontiguousarray(np.concatenate(cols, axis=1))
        bqk = np.concatenate(
            [b_attn[BLK[kind] * C + hs[lh] * DH:BLK[kind] * C + (hs[lh] + 1) * DH]
             for kind, lh in QK_ORDER]
        ).reshape(NQK, 1)
        wvp = np.zeros((C, NVP), np.float32)
        bvrow = np.zeros((1, NVP), np.float32)
        for lh in range(HPC):
            wvp[:, 65 * lh:65 * lh + DH] = \
                W_attn[:, 2 * C + hs[lh] * DH:2 * C + (hs[lh] + 1) * DH]
            bvrow[0, 65 * lh:65 * lh + DH] = \
                b_attn[2 * C + hs[lh] * DH:2 * C + (hs[lh] + 1) * DH]
            bvrow[0, 65 * lh + DH] = 1.0
        wp = np.ascontiguousarray(
            np.concatenate([W_proj[h * DH:(h + 1) * DH, :] for h in hs], axis=0)
        )
        xTc = np.ascontiguousarray(x[b].T)
        in_maps.append({
            "xT": xTc,
            "wqk": wqk,
            "bqk": np.ascontiguousarray(bqk),
            "wvp": wvp,
            "bvrow": bvrow,
            "wp": wp,
        })
    return in_maps


def unshard(per_core_outT, b_proj):
    t = per_core_outT[0].shape[1]
    out = np.zeros((B, t, C), np.float32)
    for c in range(NCORES):
        out[c // 4] += per_core_outT[c].T
    out += np.asarray(b_proj, np.float32)[None, None, :]
    return out


def kernel(x, W_attn, b_attn, W_proj, b_proj, **run_kwargs):
    nc = get_nc(T_FULL)
    in_maps = make_in_maps(x, W_attn, b_attn, W_proj)
    res = None
    last_err = None
    for attempt in range(3):
        try:
            res = run_bass_kernel_spmd(nc, in_maps,
                                       core_ids=list(range(NCORES)),
                                       **run_kwargs)
            break
        except Exception as e:  # transient NRT_EXEC_UNIT_UNRECOVERABLE etc.
            last_err = e
    if res is None:
        raise last_err
    outs = [res.results[c]["outT"] for c in range(NCORES)]
    out = unshard(outs, b_proj)
    return out


# revision 24
# speedup vs baseline: 1.4167x; 1.0313x over previous
"""Causal self-attention (B=2, T=4096, C=768, H=12) on 8 TRN2 NeuronCores.

Sharding: core c -> batch c//4, heads 3*(c%4) .. 3*(c%4)+2.  Each core is
fully independent (no collectives): it computes qkv for its 3 heads from
x[b], runs causal flash attention, and produces the partial output
projection outT = (Y_heads @ W_proj[rows]).T of shape [C, T].  The host
sums the 4 per-batch partials, transposes, and adds b_proj.

Engine-balanced design (ACT exp ~235us is the floor; everything else
hides under it):
  - PE: S^T pairs in disjoint row-groups (base-0/base-64) AND PV split
    into two 64-contraction halves (rows 0-63 -> ytA bank, 64-127 -> ytB
    bank) that also run concurrently.  Head-major loop so only 2 yt PSUM
    banks live; next-group QKV and prev-group proj interleave as filler.
  - ACT: only the 216 exp activations.
  - DVE: qkT bias-adds, vnat drains (+v-bias fold), ytA+ytB merge,
    reciprocal, normalize-mul, proj drains.
  - Pool: causal masks via in-place affine_select on p2, reciprocal
    partition-broadcast.
  - DMA: half-swapped qkT duplicates via SBUF->SBUF copies.
"""

import os
import sys

import numpy as np

for _p in ("/opt/trn_rl_repo", "/root/.axon_site/_ro/trn_rl_repo"):
    if os.path.isdir(_p) and _p not in sys.path:
        sys.path.insert(0, _p)

from contextlib import ExitStack

import concourse.bacc as bacc
import concourse.bass as bass
import concourse.mybir as mybir
import concourse.tile as tile
from concourse.bass_utils import run_bass_kernel_spmd

F32 = mybir.dt.float32
F32R = mybir.dt.float32r
EXP = mybir.ActivationFunctionType.Exp
IS_GE = mybir.AluOpType.is_ge

B, T_FULL, C = 2, 4096, 768
H, DH = 12, 64
HPC = 3                      # heads per core
NCORES = 8
P = 128
QG = 512                     # query-group span (free dim of S^T tiles)
KT = 128                     # key tile (partition dim of S^T tiles)
NQK = 6 * DH                 # 384 rows of qkT (q,k for 3 heads)
NVP = 256                    # padded v matmul width: v0|1|v1|1|v2|1|zeros
VW = 3 * 65                  # = 195 columns of vnat actually kept
SCALE = 1.0 / np.sqrt(DH)

# qkT row layout as (block, local_head) pairs of 64 rows each
QK_ORDER = [("q", 0), ("q", 1), ("k", 0), ("k", 1), ("q", 2), ("k", 2)]
BLK = {"q": 0, "k": 1, "v": 2}


def r32(ap):
    return ap.bitcast(F32R)


def build_nc(t=T_FULL, debug=False):
    ng = t // QG             # query groups
    nc = bacc.Bacc(None, target_bir_lowering=False)
    dbg = {}
    if debug:
        for name, shape in [("d_qkt0", [P, t]), ("d_qkd0", [P, t]),
                            ("d_vn0", [P, VW]), ("d_bvt", [P, NVP]),
                            ("d_p2", [P, 2 * QG]), ("d_rb", [DH, QG])]:
            dbg[name] = nc.declare_dram_parameter(name, shape, F32,
                                                  isOutput=True)
        nc._dbg_tensors = list(dbg)
    xT = nc.declare_dram_parameter("xT", [C, t], F32, isOutput=False)
    wqk = nc.declare_dram_parameter("wqk", [C, NQK], F32, isOutput=False)
    bqk = nc.declare_dram_parameter("bqk", [NQK, 1], F32, isOutput=False)
    wvp = nc.declare_dram_parameter("wvp", [C, NVP], F32, isOutput=False)
    bvrow = nc.declare_dram_parameter("bvrow", [1, NVP], F32, isOutput=False)
    wp = nc.declare_dram_parameter("wp", [HPC * DH, C], F32, isOutput=False)
    outT = nc.declare_dram_parameter("outT", [C, t], F32, isOutput=True)

    with tile.TileContext(nc) as tc, ExitStack() as ctx:
        const = ctx.enter_context(tc.tile_pool(name="const", bufs=1))
        qkp = ctx.enter_context(tc.tile_pool(name="qk", bufs=1))
        vp = ctx.enter_context(tc.tile_pool(name="vn", bufs=1))
        xpool = ctx.enter_context(tc.tile_pool(name="xin", bufs=1))
        ppool = ctx.enter_context(tc.tile_pool(name="pp", bufs=3))
        ytsb = ctx.enter_context(tc.tile_pool(name="ytsb", bufs=2))
        ytssb = ctx.enter_context(tc.tile_pool(name="ytssb", bufs=2))
        rrp = ctx.enter_context(tc.tile_pool(name="rr", bufs=2))
        osb = ctx.enter_context(tc.tile_pool(name="osb", bufs=2))
        spsum = ctx.enter_context(tc.tile_pool(name="sps", bufs=2, space="PSUM"))
        ytps = ctx.enter_context(tc.tile_pool(name="ytps", bufs=2, space="PSUM"))
        aux = ctx.enter_context(tc.tile_pool(name="aux", bufs=2, space="PSUM"))

        # ---- constants -------------------------------------------------
        # DMA order is the critical-path order: the k-th qk matmul of the
        # prologue only needs wqk[k] and x0[k], so interleave them and
        # defer everything not needed until later in the stream.
        wqk_sb, wvp_sb, x0_sb = [], [], []
        b_sb = []
        bvr_sb = const.tile([1, NVP], F32, tag="bvr", name="bvr")
        for k in range(6):
            tl = const.tile([P, NQK], F32R, tag=f"wqk{k}", name=f"wqk{k}")
            nc.sync.dma_start(tl[:], r32(wqk[k * P:(k + 1) * P, :]))
            wqk_sb.append(tl)
            tx = xpool.tile([P, QG], F32R, tag=f"x{k}", name=f"x{k}")
            nc.sync.dma_start(tx[:], r32(xT[k * P:(k + 1) * P, 0:QG]))
            x0_sb.append(tx)
            if k == 0:
                for m in range(3):
                    tb = const.tile([P, 1], F32, tag=f"bq{m}", name=f"bq{m}")
                    nc.sync.dma_start(tb[:], bqk[m * P:(m + 1) * P, :])
                    b_sb.append(tb)
                nc.sync.dma_start(bvr_sb[:], bvrow[:, :])
        for k in range(6):
            tv = const.tile([P, NVP], F32R, tag=f"wvp{k}", name=f"wvp{k}")
            nc.sync.dma_start(tv[:], r32(wvp[k * P:(k + 1) * P, :]))
            wvp_sb.append(tv)
        # v-bias row (with 1.0 in the ones columns), broadcast to all
        # partitions once: vnat drain adds it, so PV emits (Y + bv*denom)
        # rows plus the raw denominator, and the normalize-divide yields
        # Y/denom + bv with no separate bias op.
        bv_tile = const.tile([P, NVP], F32, tag="bvt", name="bvt")
        nc.gpsimd.partition_broadcast(bv_tile[:], bvr_sb[:])
        wp0 = const.tile([P, C], F32R, tag="wp0", name="wp0")
        wp1 = const.tile([DH, C], F32R, tag="wp1", name="wp1")
        nc.sync.dma_start(wp0[:], r32(wp[0:P, :]))
        nc.sync.dma_start(wp1[:], r32(wp[P:P + DH, :]))

        # ---- persistent qkT / v storage --------------------------------
        # qkT row layout: A=[q0|q1] B=[k0|k1] C=[q2|k2], plus half-swapped
        # duplicates A'=[q1|q0] B'=[k1|k0] C'=[k2|q2] so every head has its
        # (q, k) pair available at base partition 0 AND base partition 64.
        qkt = [qkp.tile([P, t], F32R, tag=f"qkt{i}", name=f"qkt{i}")
               for i in range(3)]          # A, B, C
        qkd = [qkp.tile([P, t], F32R, tag=f"qkd{i}", name=f"qkd{i}")
               for i in range(3)]          # A', B', C'
        A, Bt, Ct = qkt
        Ad, Bd, Cd = qkd
        qk0 = [(A, Bt), (Ad, Bd), (Ct, Cd)]      # base-0 (q_tile, k_tile)
        qk64 = [(Ad, Bd), (A, Bt), (Cd, Ct)]     # base-64 (q_tile, k_tile)
        # v in natural orientation: per 128-row t-tile, cols h*65..h*65+64
        # hold [v_h | ones]
        vnat = [vp.tile([P, VW], F32R, tag=f"vn{j}", name=f"vn{j}")
                for j in range(t // P)]

        def v_ap(h, j):
            return vnat[j][:, 65 * h:65 * h + DH + 1]

        # ---- per-group QKV emission (used as PE filler) ----------------
        def emit_qkv_unit(g, u):
            """Emit filler unit u (0..7) of query group g's QKV work."""
            gs = slice(g * QG, (g + 1) * QG)
            if u == 0:
                if g == 0:
                    xtiles[0] = x0_sb       # preloaded with the constants
                    return
                xk = []
                for k in range(6):
                    tl = xpool.tile([P, QG], F32R, tag=f"x{k}", name=f"x{k}")
                    nc.sync.dma_start(tl[:], r32(xT[k * P:(k + 1) * P, gs]))
                    xk.append(tl)
                xtiles[g % 2] = xk
                return
            xk = xtiles[g % 2]
            if u <= 3:
                m = u - 1           # qkT m-tile (A, B, C)
                ps = aux.tile([P, QG], F32, tag="aux", name="qkps")
                for k in range(6):
                    nc.tensor.matmul(ps[:], wqk_sb[k][:, m * P:(m + 1) * P],
                                     xk[k][:], start=(k == 0), stop=(k == 5))
                nc.vector.tensor_scalar_add(qkt[m][:, gs], ps[:], b_sb[m][:])
                # half-swapped duplicate via SBUF->SBUF DMA (frees ACT)
                nc.sync.dma_start(qkd[m][DH:, gs], qkt[m][:DH, gs])
                nc.sync.dma_start(qkd[m][:DH, gs], qkt[m][DH:, gs])
            else:
                ti = u - 4          # v t-tile within the group (0..3)
                j = 4 * g + ti
                ps = aux.tile([P, NVP], F32, tag="aux", name="vnps")
                for k in range(6):
                    nc.tensor.matmul(ps[:], xk[k][:, ti * P:(ti + 1) * P],
                                     wvp_sb[k][:], start=(k == 0), stop=(k == 5))
                nc.vector.tensor_add(vnat[j][:], ps[:, :VW], bv_tile[:, :VW])

        N_QKV_UNITS = 8  # 1 dma + 3 qk + 4 v

        def emit_proj_unit(g, cm, y0, y1):
            """Output projection for column-tile cm of group g."""
            op = aux.tile([P, QG], F32, tag="aux", name="oo")
            nc.tensor.matmul(
                op[:], wp0[:, cm * P:(cm + 1) * P], y0[:],
                start=True, stop=False,
            )
            nc.tensor.matmul(
                op[:], wp1[:, cm * P:(cm + 1) * P], y1[:],
                start=False, stop=True,
            )
            ob = osb.tile([P, QG], F32, tag="ob", name="ob")
            nc.vector.tensor_copy(ob[:], op[:])
            nc.sync.dma_start(
                outT[cm * P:(cm + 1) * P, g * QG:(g + 1) * QG], ob[:]
            )

        def emit_S(g, h, pr, npairs):
            """S^T pair + exp (+mask); returns the pending-PV job."""
            gs = slice(g * QG, (g + 1) * QG)
            q0t, k0t = qk0[h]
            q64t, k64t = qk64[h]
            j0, j1 = 2 * pr, 2 * pr + 1
            s2 = spsum.tile([P, 2 * QG], F32, tag="s", name="s")
            # two k-tiles in disjoint PE row-groups (base 0 / base 64) ->
            # the array runs them concurrently
            nc.tensor.matmul(
                s2[:, :QG],
                k0t[0:DH, j0 * P:(j0 + 1) * P],
                q0t[0:DH, gs],
                start=True, stop=True,
            )
            nc.tensor.matmul(
                s2[:, QG:],
                k64t[DH:P, j1 * P:(j1 + 1) * P],
                q64t[DH:P, gs],
                start=True, stop=True,
            )
            p2 = ppool.tile([P, 2 * QG], F32R, tag="p", name="p")
            nc.scalar.activation(p2[:], s2[:], EXP, scale=float(SCALE))
            if pr >= npairs - 2:
                # causal mask for the 2 diagonal k-tiles, in-place on the
                # Pool engine: keep col q of row k of half j iff
                # (g*512 + q) - (j*128 + k) >= 0.
                nc.gpsimd.affine_select(
                    out=p2[:],
                    in_=p2[:],
                    compare_op=IS_GE,
                    fill=0.0,
                    base=g * QG - j0 * KT,
                    pattern=[[-KT, 2], [1, QG]],
                    channel_multiplier=-1,
                )
            if debug and g == 0 and h == 0 and pr == npairs - 1:
                nc.sync.dma_start(dbg["d_p2"][:, :], p2[:].bitcast(F32))
            return (g, h, pr, npairs, p2)

        cur_yt = [None]          # live yt PSUM accumulator

        def emit_PV(job, yt0, yt1):
            """PV pair for a pending job; norm chain at head end."""
            g, h, pr, npairs, p2 = job
            first, last = (pr == 0), (pr == npairs - 1)
            j0, j1 = 2 * pr, 2 * pr + 1
            if first:
                cur_yt[0] = ytps.tile([DH + 1, QG], F32, tag="yt",
                                      name="yt")
            yt = cur_yt[0]
            nc.tensor.matmul(yt[:], v_ap(h, j0), p2[:, :QG],
                             start=first, stop=False)
            nc.tensor.matmul(yt[:], v_ap(h, j1), p2[:, QG:],
                             start=False, stop=last)
            if not last:
                return
            # head-end normalization: divide rows 0-63 by the denominator
            # row (row 64), all off-PE.  reciprocal_approx_fast requires a
            # partition-base-0 SBUF input on hardware: bounce the
            # denominator row into its own [1, QG] tile first.
            d_f = rrp.tile([1, QG], F32, tag="df", name="df")
            nc.vector.tensor_copy(d_f[:], yt[DH:DH + 1, :])
            r_f = rrp.tile([1, QG], F32, tag="rf", name="rf")
            nc.vector.reciprocal_approx_fast(r_f[:], d_f[:])
            Rb = rrp.tile([DH, QG], F32, tag="Rb", name="Rb")
            nc.gpsimd.partition_broadcast(Rb[:], r_f[:])
            dest = yt0[DH * h:DH * (h + 1), :] if h < 2 else yt1[:]
            nc.vector.tensor_mul(dest, yt[:DH, :], Rb[:])
            if debug and g == 0 and h == 0:
                nc.sync.dma_start(dbg["d_rb"][:, :], Rb[:])

        xtiles = [None, None]
        # prologue: group 0's QKV
        for u in range(N_QKV_UNITS):
            emit_qkv_unit(0, u)

        # ---- fused attention, software-pipelined: PV trails S by DEPTH
        # ---- pair-slots (continuously across heads and groups) so the
        # ---- exp+mask latency never stalls the PE.  QKV of group g+1 and
        # ---- proj of group g-1 weave in as PE filler.
        DEPTH = 2
        pend = []                # jobs with S emitted, PV outstanding
        yts_of = {}              # group -> (yt0, yt1)
        prev_y = None
        for g in range(ng):
            npairs = 2 * (g + 1)
            yt0 = ytsb.tile([P, QG], F32R, tag="yt0", name="yt0")
            yt1 = ytsb.tile([DH, QG], F32R, tag="yt1", name="yt1")
            yts_of[g] = (yt0, yt1)
            # filler: next group's QKV interleaved with prev group's proj
            fill = []
            if g + 1 < ng:
                fill = [("qkv", g + 1, u) for u in range(N_QKV_UNITS)]
            if prev_y is not None:
                pf = [("proj", g - 1, cm) for cm in range(6)]
                # weave proj after the first few QKV units (prev group's
                # normalization output must be ready)
                fill = fill[:3] + [x for pair in zip(fill[3:], pf)
                                   for x in pair] + pf[len(fill[3:]):]
            nf_total = len(fill)
            n_att = npairs * HPC
            next_fill = 0
            ui = 0
            for h in range(HPC):
                for pr in range(npairs):
                    pend.append(emit_S(g, h, pr, npairs))
                    if len(pend) > DEPTH:
                        jb = pend.pop(0)
                        emit_PV(jb, *yts_of[jb[0]])
                    ui += 1
                    while fill and next_fill <= ui * nf_total // n_att:
                        kind, gg, u = fill[0]
                        if kind == "proj" and any(j[0] < g for j in pend):
                            # prev group's final norm not yet emitted
                            break
                        fill.pop(0)
                        if kind == "qkv":
                            emit_qkv_unit(gg, u)
                        else:
                            emit_proj_unit(gg, u, *prev_y)
                        next_fill += 1
            while fill:
                kind, gg, u = fill.pop(0)
                if kind == "qkv":
                    emit_qkv_unit(gg, u)
                else:
                    emit_proj_unit(gg, u, *prev_y)
            prev_y = (yt0, yt1)
        # epilogue: drain the pipeline, then the last group's projection
        while pend:
            jb = pend.pop(0)
            emit_PV(jb, *yts_of[jb[0]])
        for cm in range(6):
            emit_proj_unit(ng - 1, cm, *prev_y)
        if debug:
            nc.sync.dma_start(dbg["d_qkt0"][:, :], qkt[0][:].bitcast(F32))
            nc.sync.dma_start(dbg["d_qkd0"][:, :], qkd[0][:].bitcast(F32))
            nc.sync.dma_start(dbg["d_vn0"][:, :], vnat[0][:].bitcast(F32))
            nc.sync.dma_start(dbg["d_bvt"][:, :], bv_tile[:])
    nc.compile()
    return nc


_NC_CACHE = {}


def get_nc(t=T_FULL):
    if t not in _NC_CACHE:
        _NC_CACHE[t] = build_nc(t)
    return _NC_CACHE[t]


def make_in_maps(x, W_attn, b_attn, W_proj):
    x = np.ascontiguousarray(np.asarray(x, np.float32))
    W_attn = np.asarray(W_attn, np.float32)
    b_attn = np.asarray(b_attn, np.float32)
    W_proj = np.asarray(W_proj, np.float32)
    in_maps = []
    for c in range(NCORES):
        b = c // 4
        hs = [3 * (c % 4) + i for i in range(HPC)]
        cols = [W_attn[:, BLK[kind] * C + hs[lh] * DH:
                       BLK[kind] * C + (hs[lh] + 1) * DH]
                for kind, lh in QK_ORDER]
        wqk = np.ascontiguousarray(np.concatenate(cols, axis=1))
        bqk = np.concatenate(
            [b_attn[BLK[kind] * C + hs[lh] * DH:BLK[kind] * C + (hs[lh] + 1) * DH]
             for kind, lh in QK_ORDER]
        ).reshape(NQK, 1)
        wvp = np.zeros((C, NVP), np.float32)
        bvrow = np.zeros((1, NVP), np.float32)
        for lh in range(HPC):
            wvp[:, 65 * lh:65 * lh + DH] = \
                W_attn[:, 2 * C + hs[lh] * DH:2 * C + (hs[lh] + 1) * DH]
            bvrow[0, 65 * lh:65 * lh + DH] = \
                b_attn[2 * C + hs[lh] * DH:2 * C + (hs[lh] + 1) * DH]
            bvrow[0, 65 * lh + DH] = 1.0
        wp = np.ascontiguousarray(
            np.concatenate([W_proj[h * DH:(h + 1) * DH, :] for h in hs], axis=0)
        )
        xTc = np.ascontiguousarray(x[b].T)
        in_maps.append({
            "xT": xTc,
            "wqk": wqk,
            "bqk": np.ascontiguousarray(bqk),
            "wvp": wvp,
            "bvrow": bvrow,
            "wp": wp,
        })
    return in_maps


def unshard(per_core_outT, b_proj):
    t = per_core_outT[0].shape[1]
    out = np.zeros((B, t, C), np.float32)
    for c in range(NCORES):
        out[c // 4] += per_core_outT[c].T
    out += np.asarray(b_proj, np.float32)[None, None, :]
    return out


def kernel(x, W_attn, b_attn, W_proj, b_proj, **run_kwargs):
    nc = get_nc(T_FULL)
    in_maps = make_in_maps(x, W_attn, b_attn, W_proj)
    res = None
    last_err = None
    for attempt in range(3):
        try:
            res = run_bass_kernel_spmd(nc, in_maps,
                                       core_ids=list(range(NCORES)),
                                       **run_kwargs)
            break
        except Exception as e:  # transient NRT_EXEC_UNIT_UNRECOVERABLE etc.
            last_err = e
    if res is None:
        raise last_err
    outs = [res.results[c]["outT"] for c in range(NCORES)]
    out = unshard(outs, b_proj)
    return out
